# revision 2
# baseline (speedup 1.0000x reference)
"""Trainium2 Bass kernel for nn_KLRS_87290915324268 (segment_reduce CBCE loss).

Math (per reference):
  lse_i = log(sum_j exp(x_ij));  nll_i = lse_i - x[i, t_i]
  sums_c = w_c * segsum(nll, t); counts_c = segsum(1, t)
  means = sums / max(counts,1); p = exp(min(means-0.5, 2)); out = sum(p*means)/N

Device strategy (data-parallel over 8 cores, 16384 rows each, fp16 x):
  Per 128-row tile [128 rows(part), 1000 cls(free)]:
    exp+rowsum, split across two engines for balance:
      ACT tiles: E = Exp(x) fp16 with fused accum_out -> se (exact exp)
      DVE tiles: E = bitcast_bf16(int16(x*A + B))   (Schraudolph approx exp,
                 tensor_scalar 4x mode), then tensor_scalar rowsum w/ accum_out
    Per 8 tiles: lse8 = Ln(se8 * exp(-7.4))  (one batched ACT Ln)
    Segment-sum via factorized onehot (c = 32a + b):
      lhA = (iota32 == a_j) * lse_j   [128, 32]  (one DVE tensor_scalar, 4x)
      rhB = (iota32 == b_j)           [128, 32]
      psum[32,32] += lhA.T @ rhB      (PE, free dim 32 -> ~25ns)
  Output per core: out32[a, b] = sum over rows with t=32a+b of (lse - 7.4)
Host: x_t gather + counts + per-class xt sums from the original fp32 input,
  then the [C]-sized exp-reweight epilogue (all O(N) / O(C), no O(N*C) work).
"""

import numpy as np
from contextlib import ExitStack

import concourse.bacc as bacc
import concourse.tile as tile
import concourse.mybir as mybir
from concourse.bass_utils import run_bass_kernel_spmd

# The act-table-load inserter picks the first table set containing each
# activation function; Exp and Ln only live together in
# natural_log_exp_and_others.  Strip them from every other set (dict order
# preserved) so the pass hoists a single combined load instead of
# re-loading per Exp<->Ln alternation (~2.7us each).
_orig_get_act_tables = bacc.get_activation_tables


def _combined_act_tables(arch):
    tabs = _orig_get_act_tables(arch)
    AF = mybir.ActivationFunctionType
    for name, s in tabs.items():
        if name != "natural_log_exp_and_others":
            s.discard(AF.Exp)
            s.discard(AF.Ln)
    return tabs


bacc.get_activation_tables = _combined_act_tables

P = 128          # partitions
C = 1000         # classes
NCORES = 8
N_TOTAL = 131072
N_CORE = N_TOTAL // NCORES   # 16384
NT = N_CORE // P             # 128 row-tiles per core
TPD = 4                      # row-tiles per DMA (4 * 256KB fp16 = 1MB chunks)
GRP = 8                      # tiles per batched Ln

LSE_SHIFT = 7.4
S_SCALE = float(np.float32(np.exp(-LSE_SHIFT)))   # Ln scale immediate
CTILDE = 0.055
SCHRA_A = float(np.float32(128.0 / np.log(2.0)))
SCHRA_B = float(np.float32(16256.0 - 128.0 * CTILDE))
ACT_MOD = 7     # tiles with (j % ACT_MOD) < ACT_CNT take the ACT exp path
ACT_CNT = 4

_cache = {}


def build_nc(nt=NT, tpd=TPD, reps=1, reps_loop=1, act_mod=ACT_MOD,
             act_cnt=ACT_CNT):
    nc = bacc.Bacc(None, target_bir_lowering=False)
    f32 = mybir.dt.float32
    f16 = mybir.dt.float16
    bf16 = mybir.dt.bfloat16
    i16 = mybir.dt.int16
    AF = mybir.ActivationFunctionType
    OP = mybir.AluOpType

    x = nc.dram_tensor("x", [nt * P, C], f16, kind="ExternalInput")
    at = nc.dram_tensor("at", [P, nt], f32, kind="ExternalInput")
    bt = nc.dram_tensor("bt", [P, nt], f32, kind="ExternalInput")
    iota = nc.dram_tensor("iota", [P, 32], bf16, kind="ExternalInput")
    out = nc.dram_tensor("out", [32, 32], f32, kind="ExternalOutput")

    with tile.TileContext(nc) as tc, ExitStack() as ctx:
        xp = ctx.enter_context(tc.tile_pool(name="xp", bufs=3))
        ep = ctx.enter_context(tc.tile_pool(name="ep", bufs=2))   # int16 schraudolph
        scp = ctx.enter_context(tc.tile_pool(name="scp", bufs=2))  # rowsum scratch
        esp = ctx.enter_context(tc.tile_pool(name="esp", bufs=2))  # ACT exp scratch
        sep = ctx.enter_context(tc.tile_pool(name="sep", bufs=3))  # se8 groups
        lsp = ctx.enter_context(tc.tile_pool(name="lsp", bufs=3))  # lse8 groups
        ohp = ctx.enter_context(tc.tile_pool(name="ohp", bufs=4))  # lhA / rhB
        sgp = ctx.enter_context(tc.tile_pool(name="sgp", bufs=1))
        psp = ctx.enter_context(tc.tile_pool(name="psp", bufs=1, space="PSUM"))

        iota_sb = sgp.tile([P, 32], bf16)
        nc.sync.dma_start(out=iota_sb[:], in_=iota[:])
        at_sb = sgp.tile([P, nt], f32)
        nc.sync.dma_start(out=at_sb[:], in_=at[:])
        bt_sb = sgp.tile([P, nt], f32)
        nc.sync.dma_start(out=bt_sb[:], in_=bt[:])

        ps = psp.tile([32, 32], f32)

        # device row (nd, p, t) = host row (nd*P + p)*tpd + t
        xv = x[:].rearrange("(nd p t) c -> nd p t c", p=P, t=tpd)

        n_grp = nt // GRP
        dma_per_grp = GRP // tpd

        def rep_body():
            for g in range(n_grp):
                se8 = sep.tile([P, GRP], f32)
                xts = []
                for d in range(dma_per_grp):
                    nd = g * dma_per_grp + d
                    xt_ = xp.tile([P, tpd, C], f16)
                    nc.sync.dma_start(out=xt_[:], in_=xv[nd, :, :, :])
                    xts.append(xt_)
                # phase 1: exp + rowsum for the 8 tiles of this group
                for d in range(dma_per_grp):
                    xt_ = xts[d]
                    for t in range(tpd):
                        j = (g * dma_per_grp + d) * tpd + t
                        col = j % GRP
                        if (j % act_mod) < act_cnt:
                            es = esp.tile([P, C], f16)
                            nc.scalar.activation(out=es[:], in_=xt_[:, t, :],
                                                 func=AF.Exp,
                                                 accum_out=se8[:, col:col + 1])
                        else:
                            ei = ep.tile([P, C], i16)
                            nc.vector.tensor_scalar(out=ei[:], in0=xt_[:, t, :],
                                                    scalar1=SCHRA_A,
                                                    scalar2=SCHRA_B,
                                                    op0=OP.mult, op1=OP.add)
                            h1 = scp.tile([P, C // 2], bf16)
                            nc.vector.tensor_tensor(
                                out=h1[:],
                                in0=ei[:, 0:C // 2].bitcast(bf16),
                                in1=ei[:, C // 2:C].bitcast(bf16),
                                op=OP.add)
                            sc = scp.tile([P, C // 2], bf16)
                            nc.vector.tensor_scalar(out=sc[:], in0=h1[:],
                                                    scalar1=1.0, scalar2=None,
                                                    op0=OP.mult, op1=OP.add,
                                                    accum_out=se8[:, col:col + 1])
                # phase 2: batched Ln -> lse - LSE_SHIFT
                lse8 = lsp.tile([P, GRP], f32)
                nc.scalar.activation(out=lse8[:], in_=se8[:], func=AF.Ln,
                                     scale=S_SCALE)
                # phase 3: factorized segment matmul
                for q in range(GRP):
                    j = g * GRP + q
                    lhA = ohp.tile([P, 32], bf16)
                    nc.vector.tensor_scalar(out=lhA[:], in0=iota_sb[:],
                                            scalar1=at_sb[:, j:j + 1],
                                            scalar2=lse8[:, q:q + 1],
                                            op0=OP.is_equal, op1=OP.mult)
                    rhB = ohp.tile([P, 32], bf16)
                    nc.vector.tensor_scalar(out=rhB[:], in0=iota_sb[:],
                                            scalar1=bt_sb[:, j:j + 1],
                                            scalar2=None, op0=OP.is_equal)
                    nc.tensor.matmul(out=ps[:], lhsT=lhA[:], rhs=rhB[:],
                                     start=(j == 0), stop=(j == nt - 1))

        if reps_loop > 1:
            with tc.For_i(0, reps_loop, 1):
                rep_body()
        else:
            for _ in range(reps):
                rep_body()

        ob = sgp.tile([32, 32], f32)
        nc.vector.tensor_copy(out=ob[:], in_=ps[:])
        nc.sync.dma_start(out=out[:], in_=ob[:])

    nc.compile()
    return nc


def _get_nc():
    if "nc" not in _cache:
        _cache["nc"] = build_nc()
    return _cache["nc"]


def _make_in_maps(x16, target):
    import ml_dtypes
    iota_bf16 = np.ascontiguousarray(
        np.broadcast_to(np.arange(32, dtype=ml_dtypes.bfloat16), (P, 32)))
    in_maps = []
    for k in range(NCORES):
        xs = x16[k * N_CORE:(k + 1) * N_CORE]
        tg = target[k * N_CORE:(k + 1) * N_CORE].astype(np.int64)
        # at[p, j] with j = nd*TPD + t ; device row (nd,p,t) = host row
        # nd*512 + p*4 + t
        tg3 = tg.reshape(NT // TPD, P, TPD)
        a3 = (tg3 >> 5).astype(np.float32).transpose(1, 0, 2).reshape(P, NT)
        b3 = (tg3 & 31).astype(np.float32).transpose(1, 0, 2).reshape(P, NT)
        in_maps.append({
            "x": xs,
            "at": np.ascontiguousarray(a3),
            "bt": np.ascontiguousarray(b3),
            "iota": iota_bf16,
        })
    return in_maps


def _epilogue(outs, output_f32, target, cls_weights, lam, N):
    # device gave sum over rows in class c of (ln(sumexp) - LSE_SHIFT)
    sum_lse_dev = np.zeros(1024, np.float64)
    for o in outs:
        sum_lse_dev += o.reshape(1024).astype(np.float64)
    sum_lse_dev = sum_lse_dev[:C]
    counts = np.bincount(target, minlength=C).astype(np.float64)
    ln_S = np.log(np.float64(np.float32(S_SCALE)))
    sum_lse = sum_lse_dev - counts * ln_S
    xt = np.take_along_axis(output_f32, target[:, None], axis=1)[:, 0]
    sum_xt = np.bincount(target, weights=xt.astype(np.float64), minlength=C)
    sums = np.asarray(cls_weights, np.float64) * (sum_lse - sum_xt)
    if lam >= 200:
        return np.float32(sums.sum() / N)
    means = sums / np.maximum(counts, 1.0)
    p = np.exp(np.minimum((means - 0.5) / lam, 2.0))
    return np.float32((p * means).sum() / N)


def run_cores(output, target, trace=False):
    nc = _get_nc()
    x16 = np.asarray(output).astype(np.float16)
    in_maps = _make_in_maps(x16, np.asarray(target))
    res = run_bass_kernel_spmd(nc, in_maps, core_ids=list(range(NCORES)),
                               trace=trace)
    return res


def kernel(output, target, cls_weights, myLambda):
    output = np.asarray(output, dtype=np.float32)
    target = np.asarray(target).astype(np.int64)
    lam = int(np.asarray(myLambda))
    res = run_cores(output, target, trace=False)
    outs = [r["out"] for r in res.results]
    return _epilogue(outs, output, target, cls_weights, lam, output.shape[0])
